# revision 12
# baseline (speedup 1.0000x reference)
"""Distributed causal multi-head attention (GPT-2 style Attention block)
for 8 Trainium2 NeuronCores.

Problem (hardcoded shapes): B=2, S=2048, D=1024, H=16 heads, Dh=64, f32.
reference computes:
    qkv = x @ w_attn + b_attn ; split q,k,v ; heads
    w = softmax(causal_mask(q k^T / 8))
    a = (w v) merged @ w_proj + b_proj
    present = stack(k, v)   # [2, B, H, S, Dh]

Sharding: data + head parallel. Core c handles batch b=c//4 and heads
H_c = [4*(c%4), 4*(c%4)+4). Each core:
  - computes q^T,k^T (head-dim-major) and v (seq-major) for its heads
  - full causal attention for its 4 heads over all S (identical static
    structure on every core -> one SPMD graph)
  - c_proj partial product with its 256 rows of w_proj
  - ReduceScatter(add) over its 4-core batch group, chunked by 512-row
    blocks of S so comm overlaps the remaining attention compute.
Matmul operands are bf16 (fast weight load + 2 elem/cycle streaming);
accumulation stays f32 in PSUM; softmax statistics stay f32.

kernel(**inputs) takes the FULL unsharded inputs and returns the full
(a, present) pair like the reference.
"""

import numpy as np

P = 128
B, S, D = 2, 2048, 1024
H = 16
Dh = 64
HPC = 4  # heads per core
NQ = 512  # q-block width
NB = S // NQ  # 4 q blocks
KT = S // P  # 16 k-tiles
KD = D // P  # 8 contraction tiles over D
GROUPS = [[0, 1, 2, 3], [4, 5, 6, 7]]

_CACHE = {}


def build_kernel():
    import concourse.mybir as mybir
    import concourse.tile as tile
    from concourse import bacc

    f32 = mybir.dt.float32
    f32r = mybir.dt.float32r
    bf16 = mybir.dt.bfloat16

    nc = bacc.Bacc(None, target_bir_lowering=False, num_devices=8)

    # ---- per-core external inputs ----
    xT = nc.dram_tensor("xT", [D, S], f32, kind="ExternalInput")
    wqkv = nc.dram_tensor("wqkv", [D, 3 * HPC * Dh], f32, kind="ExternalInput")
    bqk = nc.dram_tensor("bqk", [2 * HPC * Dh, 1], f32, kind="ExternalInput")
    bvb = nc.dram_tensor("bvb", [P, HPC * Dh], f32, kind="ExternalInput")
    wp = nc.dram_tensor("wp", [HPC * Dh, D], f32, kind="ExternalInput")
    bpb = nc.dram_tensor("bpb", [P, D], bf16, kind="ExternalInput")
    cmask = nc.dram_tensor("cmask", [4, P, NQ], bf16, kind="ExternalInput")

    # ---- per-core outputs ----
    kp = nc.dram_tensor("kp", [Dh, HPC * S], bf16, kind="ExternalOutput")
    vp = nc.dram_tensor("vp", [S, HPC * Dh], bf16, kind="ExternalOutput")
    a_out = nc.dram_tensor("a_out", [NB, P, D], f32, kind="ExternalOutput")

    # ---- internal DRAM for the chunked reduce-scatter ----
    cc_in = [
        nc.dram_tensor(f"cc_in{qb}", [NQ, D], bf16, kind="Internal")
        for qb in range(NB)
    ]
    cc_out = [
        nc.dram_tensor(f"cc_out{qb}", [P, D], bf16, kind="Internal")
        for qb in range(NB)
    ]

    with tile.TileContext(nc) as tc:
        with (
            tc.tile_pool(name="res", bufs=1) as res,  # whole-kernel residents
            tc.tile_pool(name="wrk", bufs=3) as wrk,  # rotating staging
            tc.tile_pool(name="stp", bufs=4, space="PSUM") as stp,
            tc.tile_pool(name="opp", bufs=2, space="PSUM") as opp,
            tc.tile_pool(name="mpp", bufs=2, space="PSUM") as mpp,
        ):
            # ---------- whole-kernel resident tensors ----------
            mask_sb = res.tile([P, 4 * NQ], bf16, tag="mask")
            for t in range(4):
                nc.sync.dma_start(mask_sb[:, NQ * t : NQ * (t + 1)], cmask[t])
            wp_sb = []
            for k in range(2):
                wptmp = wrk.tile([P, D], f32, tag="wptmp", name="wptmp", bufs=2)
                nc.sync.dma_start(wptmp[:], wp[P * k : P * (k + 1), :])
                t = res.tile([P, D], bf16, tag=f"wp{k}", name=f"wp{k}")
                nc.vector.tensor_copy(t[:], wptmp[:])
                wp_sb.append(t)
            bqk_sb = []
            for m in range(4):
                t = res.tile([P, 1], f32, tag=f"bqk{m}", name=f"bqk{m}")
                nc.sync.dma_start(t[:], bqk[P * m : P * (m + 1), :])
                bqk_sb.append(t)
            bvb_sb = res.tile([P, HPC * Dh], f32, tag="bvb")
            nc.sync.dma_start(bvb_sb[:], bvb[:])
            bpb_sb = res.tile([P, D], bf16, tag="bpb")
            nc.sync.dma_start(bpb_sb[:], bpb[:])
            ones64f = res.tile([1, Dh], f32, tag="ones64f")
            nc.vector.memset(ones64f[:], 1.0)
            ones64 = res.tile([1, Dh], f32r, tag="ones64")
            nc.vector.tensor_copy(ones64[:], ones64f[:])
            ones4f = res.tile([P, HPC], f32, tag="ones4f")
            nc.vector.memset(ones4f[:], 1.0)

            # q^T / k^T head-dim-major, per 512-col block:
            # qT_sb[n] is [64, HPC*NQ]; head h occupies cols [h*NQ,(h+1)*NQ)
            qT_sb = [
                res.tile([Dh, HPC * NQ], bf16, tag=f"qT{n}", name=f"qT{n}")
                for n in range(NB)
            ]
            kT_sb = [
                res.tile([Dh, HPC * NQ], bf16, tag=f"kT{n}", name=f"kT{n}")
                for n in range(NB)
            ]
            # v seq-major with a ones column per head: [128, HPC*65] per s-tile
            v_sb = [
                res.tile([P, HPC * (Dh + 1)], bf16, tag=f"v{st}", name=f"v{st}")
                for st in range(KT)
            ]
            # attention output^T (d-major) for the current q block
            attnT_sb = [
                res.tile([P, NQ], bf16, tag=f"attnT{t}", name=f"attnT{t}")
                for t in range(2)
            ]

            # ---------- phase 1: qkv projections ----------
            # stream xT in 512-wide column blocks; cast everything to bf16
            with tc.tile_pool(name="ph1", bufs=2) as ph1:
                wq_sb = []
                for k in range(KD):
                    wtmp = ph1.tile(
                        [P, 3 * HPC * Dh], f32, tag="wtmp", name="wtmp", bufs=4
                    )
                    nc.sync.dma_start(wtmp[:], wqkv[P * k : P * (k + 1), :])
                    w = ph1.tile(
                        [P, 3 * HPC * Dh],
                        bf16,
                        tag=f"w{k}",
                        name=f"w{k}",
                        bufs=1,
                    )
                    nc.scalar.activation(
                        w[:], wtmp[:], mybir.ActivationFunctionType.Copy
                    )
                    wq_sb.append(w)
                for n in range(NB):
                    xb = []
                    for k in range(KD):
                        xtmp = ph1.tile(
                            [P, NQ], f32, tag="xtmp", name="xtmp", bufs=3
                        )
                        nc.sync.dma_start(
                            xtmp[:],
                            xT[P * k : P * (k + 1), NQ * n : NQ * (n + 1)],
                        )
                        t = ph1.tile(
                            [P, NQ], bf16, tag=f"x{k}", name=f"x{k}", bufs=1
                        )
                        nc.scalar.activation(
                            t[:], xtmp[:], mybir.ActivationFunctionType.Copy
                        )
                        xb.append(t)
                    # q,k d-major: m 0,1 -> q heads {0,1},{2,3}; m 2,3 -> k
                    for m in range(4):
                        dst = qT_sb[n] if m < 2 else kT_sb[n]
                        h0 = 2 * (m % 2)
                        ps = mpp.tile([P, NQ], f32, tag="mm", name="psqk")
                        for k in range(KD):
                            nc.tensor.matmul(
                                ps[:],
                                wq_sb[k][:, P * m : P * (m + 1)],
                                xb[k][:],
                                start=(k == 0),
                                stop=(k == KD - 1),
                            )
                        for j in range(2):
                            nc.scalar.activation(
                                dst[:, (h0 + j) * NQ : (h0 + j + 1) * NQ],
                                ps[Dh * j : Dh * (j + 1), :],
                                mybir.ActivationFunctionType.Identity,
                                bias=bqk_sb[m][Dh * j : Dh * (j + 1), :],
                            )
                    # v seq-major: out [128 s, 256 dv] per s-tile
                    for st in range(4 * n, 4 * n + 4):
                        ps = mpp.tile([P, HPC * Dh], f32, tag="mm", name="psv")
                        for k in range(KD):
                            nc.tensor.matmul(
                                ps[:],
                                xb[k][:, P * (st % 4) : P * (st % 4 + 1)],
                                wq_sb[k][:, 2 * HPC * Dh : 3 * HPC * Dh],
                                start=(k == 0),
                                stop=(k == KD - 1),
                            )
                        vdst = v_sb[st][:].rearrange(
                            "p (h c) -> p h c", c=Dh + 1
                        )
                        nc.vector.tensor_tensor(
                            vdst[:, :, 0:Dh],
                            ps[:].rearrange("p (h c) -> p h c", c=Dh),
                            bvb_sb[:].rearrange("p (h c) -> p h c", c=Dh),
                            mybir.AluOpType.add,
                        )
                        nc.vector.tensor_copy(
                            vdst[:, :, Dh : Dh + 1], ones4f[:]
                        )
                        # stream out the present-v part
                        nc.sync.dma_start(
                            vp[P * st : P * (st + 1), :], vdst[:, :, 0:Dh]
                        )
                    # stream out the present-k part for this block
                    nc.sync.dma_start(
                        kp[:].rearrange("d (h s) -> d h s", s=S)[
                            :, :, NQ * n : NQ * (n + 1)
                        ],
                        kT_sb[n][:].rearrange("d (h s) -> d h s", s=NQ),
                    )

            # ---------- phase 2: attention + c_proj + reduce-scatter ----------
            for qb in range(NB):
                for h in range(HPC):
                    o_ps = opp.tile([P, NQ], f32, tag="o", name="o_ps")
                    nkt = 4 * qb + 4
                    for kt in range(nkt):
                        t = kt - 4 * qb
                        # diagonal tiles only touch q columns >= 128*t
                        c0 = P * t if t > 0 else 0
                        w = NQ - c0
                        st_ps = stp.tile([P, NQ], f32, tag="st", name="st_ps")
                        nc.tensor.matmul(
                            st_ps[:, 0:w],
                            kT_sb[kt // 4][
                                :, h * NQ + P * (kt % 4) : h * NQ + P * (kt % 4 + 1)
                            ],
                            qT_sb[qb][:, h * NQ + c0 : (h + 1) * NQ],
                            start=True,
                            stop=True,
                        )
                        st_sb = wrk.tile(
                            [P, NQ], bf16, tag="stsb", name="st_sb"
                        )
                        nc.scalar.activation(
                            st_sb[:, 0:w],
                            st_ps[:, 0:w],
                            mybir.ActivationFunctionType.Exp,
                            scale=0.125,
                        )
                        if t >= 0:
                            nc.vector.tensor_tensor(
                                st_sb[:, 0:w],
                                st_sb[:, 0:w],
                                mask_sb[:, NQ * t + c0 : NQ * (t + 1)],
                                mybir.AluOpType.mult,
                            )
                        nc.tensor.matmul(
                            o_ps[0 : Dh + 1, c0:NQ],
                            v_sb[kt][:, h * (Dh + 1) : (h + 1) * (Dh + 1)],
                            st_sb[:, 0:w],
                            start=(kt == 0),
                            stop=(kt == nkt - 1),
                        )
                    # normalize: attnT[h] = o[0:64] / l  (l = row 64 of o)
                    linv = wrk.tile([1, NQ], f32r, tag="linv", name="linv")
                    with nc.allow_low_precision(reason="f32r recip, 2^-19 rel"):
                        nc.vector.reciprocal(linv[:], o_ps[Dh : Dh + 1, :])
                    b_ps = mpp.tile([P, NQ], f32, tag="mm", name="b_ps")
                    nc.tensor.matmul(
                        b_ps[0:Dh, :],
                        ones64[:],
                        linv[:],
                        start=True,
                        stop=True,
                    )
                    bc_sb = wrk.tile([Dh, NQ], f32, tag="bc", name="bc_sb")
                    nc.vector.tensor_copy(bc_sb[:], b_ps[0:Dh, :])
                    nc.vector.tensor_tensor(
                        attnT_sb[h // 2][Dh * (h % 2) : Dh * (h % 2 + 1), :],
                        o_ps[0:Dh, :],
                        bc_sb[:],
                        mybir.AluOpType.mult,
                    )
                # c_proj partial for this q block
                for m in range(4):
                    for half in range(2):
                        ps = mpp.tile([P, NQ], f32, tag="mm", name="ps_cp")
                        for kt2 in range(2):
                            nc.tensor.matmul(
                                ps[:],
                                attnT_sb[kt2][:, P * m : P * (m + 1)],
                                wp_sb[kt2][:, NQ * half : NQ * (half + 1)],
                                start=(kt2 == 0),
                                stop=(kt2 == 1),
                            )
                        ap_sb = wrk.tile([P, NQ], bf16, tag="ap", name="ap_sb")
                        nc.vector.tensor_copy(ap_sb[:], ps[:])
                        nc.sync.dma_start(
                            cc_in[qb][
                                P * m : P * (m + 1), NQ * half : NQ * (half + 1)
                            ],
                            ap_sb[:],
                        )
                nc.gpsimd.collective_compute(
                    "ReduceScatter",
                    mybir.AluOpType.add,
                    ins=[cc_in[qb][:]],
                    outs=[cc_out[qb][:]],
                    replica_groups=GROUPS,
                )
                rs_sb = wrk.tile([P, D], bf16, tag="rs", name="rs_sb", bufs=2)
                nc.sync.dma_start(rs_sb[:], cc_out[qb][:])
                ao_sb = wrk.tile([P, D], f32, tag="ao", name="ao_sb", bufs=2)
                nc.vector.tensor_tensor(
                    ao_sb[:], rs_sb[:], bpb_sb[:], mybir.AluOpType.add
                )
                nc.sync.dma_start(a_out[qb], ao_sb[:])

    nc.compile()
    return nc


def shard_inputs(x, w_attn, b_attn, w_proj, b_proj):
    """Build the 8 per-core input maps from full inputs."""
    import ml_dtypes

    bf = ml_dtypes.bfloat16
    x = np.asarray(x, dtype=np.float32)
    w_attn = np.asarray(w_attn, dtype=np.float32)
    b_attn = np.asarray(b_attn, dtype=np.float32)
    w_proj = np.asarray(w_proj, dtype=np.float32)
    b_proj = np.asarray(b_proj, dtype=np.float32)

    # causal masks for the 4 diagonal k-tiles of each 512-wide q block
    i = np.arange(P)[:, None]
    j = np.arange(NQ)[None, :]
    cmask = np.stack([(j >= (P * t + i)).astype(bf) for t in range(4)])
    bpb = np.tile(b_proj[None, :], (P, 1)).astype(bf)

    in_maps = []
    for c in range(8):
        b = c // 4
        g = c % 4
        cols = slice(HPC * Dh * g, HPC * Dh * (g + 1))  # 256 cols of this core
        wq = w_attn[:, 0 * D : 1 * D][:, cols]
        wk = w_attn[:, 1 * D : 2 * D][:, cols]
        wv = w_attn[:, 2 * D : 3 * D][:, cols]
        bq = b_attn[0 * D : 1 * D][cols]
        bk = b_attn[1 * D : 2 * D][cols]
        bv = b_attn[2 * D : 3 * D][cols]
        in_maps.append(
            {
                "xT": np.ascontiguousarray(x[b].T),
                "wqkv": np.ascontiguousarray(
                    np.concatenate([wq, wk, wv], axis=1)
                ),
                "bqk": np.concatenate([bq, bk])[:, None].astype(np.float32),
                "bvb": np.tile(bv[None, :], (P, 1)).astype(np.float32),
                "wp": np.ascontiguousarray(w_proj[cols, :]),
                "bpb": bpb,
                "cmask": cmask,
            }
        )
    return in_maps


def assemble_outputs(results):
    """results: list of 8 per-core {kp, vp, a_out} -> (a, present)."""
    a = np.empty((B, S, D), dtype=np.float32)
    k = np.empty((B, H, S, Dh), dtype=np.float32)
    v = np.empty((B, H, S, Dh), dtype=np.float32)
    for c in range(8):
        b = c // 4
        g = c % 4
        kp = np.asarray(results[c]["kp"], dtype=np.float32)  # [64, HPC*S]
        vp = np.asarray(results[c]["vp"], dtype=np.float32)  # [S, HPC*64]
        for j in range(HPC):
            k[b, HPC * g + j] = kp[:, S * j : S * (j + 1)].T
            v[b, HPC * g + j] = vp[:, Dh * j : Dh * (j + 1)]
        ao = results[c]["a_out"]  # [NB, 128, D]
        for qb in range(NB):
            r0 = NQ * qb + P * g
            a[b, r0 : r0 + P] = ao[qb]
    present = np.stack([k, v])
    return a, present


def _get_nc():
    if "nc" not in _CACHE:
        _CACHE["nc"] = build_kernel()
    return _CACHE["nc"]


def kernel(x, w_attn, b_attn, w_proj, b_proj):
    from concourse.bass_utils import run_bass_kernel_spmd

    nc = _get_nc()
    in_maps = shard_inputs(x, w_attn, b_attn, w_proj, b_proj)
    res = run_bass_kernel_spmd(nc, in_maps, core_ids=list(range(8)))
    return assemble_outputs(res.results)


# revision 13
# speedup vs baseline: 1.1183x; 1.1183x over previous
"""Distributed causal multi-head attention (GPT-2 style Attention block)
for 8 Trainium2 NeuronCores.

Problem (hardcoded shapes): B=2, S=2048, D=1024, H=16 heads, Dh=64, f32.
reference computes:
    qkv = x @ w_attn + b_attn ; split q,k,v ; heads
    w = softmax(causal_mask(q k^T / 8))
    a = (w v) merged @ w_proj + b_proj
    present = stack(k, v)   # [2, B, H, S, Dh]

Sharding: data + head parallel. Core c handles batch b=c//4 and heads
H_c = [4*(c%4), 4*(c%4)+4). Each core:
  - computes q^T,k^T (head-dim-major) and v (seq-major) for its heads
  - full causal attention for its 4 heads over all S (identical static
    structure on every core -> one SPMD graph)
  - c_proj partial product with its 256 rows of w_proj
  - ReduceScatter(add) over its 4-core batch group, chunked by 512-row
    blocks of S so comm overlaps the remaining attention compute.
Matmul operands are bf16 (fast weight load + 2 elem/cycle streaming);
accumulation stays f32 in PSUM; softmax statistics stay f32.

kernel(**inputs) takes the FULL unsharded inputs and returns the full
(a, present) pair like the reference.
"""

import numpy as np

P = 128
B, S, D = 2, 2048, 1024
H = 16
Dh = 64
HPC = 4  # heads per core
NQ = 512  # q-block width
NB = S // NQ  # 4 q blocks
KT = S // P  # 16 k-tiles
KD = D // P  # 8 contraction tiles over D
GROUPS = [[0, 1, 2, 3], [4, 5, 6, 7]]

_CACHE = {}


def build_kernel():
    import concourse.mybir as mybir
    import concourse.tile as tile
    from concourse import bacc

    f32 = mybir.dt.float32
    f32r = mybir.dt.float32r
    bf16 = mybir.dt.bfloat16

    nc = bacc.Bacc(None, target_bir_lowering=False, num_devices=8)

    # ---- per-core external inputs ----
    xT = nc.dram_tensor("xT", [D, S], f32, kind="ExternalInput")
    wqkv = nc.dram_tensor("wqkv", [D, 3 * HPC * Dh], f32, kind="ExternalInput")
    bqk = nc.dram_tensor("bqk", [2 * HPC * Dh, 1], f32, kind="ExternalInput")
    bvb = nc.dram_tensor("bvb", [P, HPC * Dh], f32, kind="ExternalInput")
    wp = nc.dram_tensor("wp", [HPC * Dh, D], f32, kind="ExternalInput")
    bpb = nc.dram_tensor("bpb", [P, D], bf16, kind="ExternalInput")
    cmask = nc.dram_tensor("cmask", [4, P, NQ], bf16, kind="ExternalInput")

    # ---- per-core outputs ----
    kp = nc.dram_tensor("kp", [Dh, HPC * S], bf16, kind="ExternalOutput")
    vp = nc.dram_tensor("vp", [S, HPC * Dh], bf16, kind="ExternalOutput")
    a_out = nc.dram_tensor("a_out", [NB, P, D], f32, kind="ExternalOutput")

    # ---- internal DRAM for the chunked reduce-scatter ----
    cc_in = [
        nc.dram_tensor(f"cc_in{qb}", [NQ, D], bf16, kind="Internal")
        for qb in range(NB)
    ]
    cc_out = [
        nc.dram_tensor(f"cc_out{qb}", [P, D], bf16, kind="Internal")
        for qb in range(NB)
    ]

    with tile.TileContext(nc) as tc:
        with (
            tc.tile_pool(name="res", bufs=1) as res,  # whole-kernel residents
            tc.tile_pool(name="wrk", bufs=3) as wrk,  # rotating staging
            tc.tile_pool(name="stp", bufs=4, space="PSUM") as stp,
            tc.tile_pool(name="opp", bufs=2, space="PSUM") as opp,
            tc.tile_pool(name="mpp", bufs=2, space="PSUM") as mpp,
        ):
            # ---------- whole-kernel resident tensors ----------
            bqk_sb = []
            for m in range(4):
                t = res.tile([P, 1], f32, tag=f"bqk{m}", name=f"bqk{m}")
                nc.sync.dma_start(t[:], bqk[P * m : P * (m + 1), :])
                bqk_sb.append(t)
            bvb_sb = res.tile([P, HPC * Dh], f32, tag="bvb")
            nc.sync.dma_start(bvb_sb[:], bvb[:])
            ones64f = res.tile([1, Dh], f32, tag="ones64f")
            nc.vector.memset(ones64f[:], 1.0)
            ones64 = res.tile([1, Dh], f32r, tag="ones64")
            nc.vector.tensor_copy(ones64[:], ones64f[:])
            ones4f = res.tile([P, HPC], f32, tag="ones4f")
            nc.vector.memset(ones4f[:], 1.0)

            # q^T / k^T head-dim-major, per 512-col block:
            # qT_sb[n] is [64, HPC*NQ]; head h occupies cols [h*NQ,(h+1)*NQ)
            qT_sb = [
                res.tile([Dh, HPC * NQ], bf16, tag=f"qT{n}", name=f"qT{n}")
                for n in range(NB)
            ]
            kT_sb = [
                res.tile([Dh, HPC * NQ], bf16, tag=f"kT{n}", name=f"kT{n}")
                for n in range(NB)
            ]
            # v seq-major with a ones column per head: [128, HPC*65] per s-tile
            v_sb = [
                res.tile([P, HPC * (Dh + 1)], bf16, tag=f"v{st}", name=f"v{st}")
                for st in range(KT)
            ]
            # attention output^T (d-major) for the current q block
            attnT_sb = [
                res.tile([P, NQ], bf16, tag=f"attnT{t}", name=f"attnT{t}")
                for t in range(2)
            ]

            # ---------- phase 1: qkv projections ----------
            # stream xT in 512-wide column blocks; cast everything to bf16
            with tc.tile_pool(name="ph1", bufs=2) as ph1:
                wq_sb = []
                for k in range(KD):
                    wtmp = ph1.tile(
                        [P, 3 * HPC * Dh], f32, tag="wtmp", name="wtmp", bufs=4
                    )
                    nc.sync.dma_start(wtmp[:], wqkv[P * k : P * (k + 1), :])
                    w = ph1.tile(
                        [P, 3 * HPC * Dh],
                        bf16,
                        tag=f"w{k}",
                        name=f"w{k}",
                        bufs=1,
                    )
                    nc.scalar.activation(
                        w[:], wtmp[:], mybir.ActivationFunctionType.Copy
                    )
                    wq_sb.append(w)
                for n in range(NB):
                    xb = []
                    for k in range(KD):
                        xtmp = ph1.tile(
                            [P, NQ], f32, tag="xtmp", name="xtmp", bufs=3
                        )
                        nc.sync.dma_start(
                            xtmp[:],
                            xT[P * k : P * (k + 1), NQ * n : NQ * (n + 1)],
                        )
                        t = ph1.tile(
                            [P, NQ], bf16, tag=f"x{k}", name=f"x{k}", bufs=1
                        )
                        nc.scalar.activation(
                            t[:], xtmp[:], mybir.ActivationFunctionType.Copy
                        )
                        xb.append(t)
                    # q,k d-major: m 0,1 -> q heads {0,1},{2,3}; m 2,3 -> k
                    for m in range(4):
                        dst = qT_sb[n] if m < 2 else kT_sb[n]
                        h0 = 2 * (m % 2)
                        ps = mpp.tile([P, NQ], f32, tag="mm", name="psqk")
                        for k in range(KD):
                            nc.tensor.matmul(
                                ps[:],
                                wq_sb[k][:, P * m : P * (m + 1)],
                                xb[k][:],
                                start=(k == 0),
                                stop=(k == KD - 1),
                            )
                        for j in range(2):
                            nc.scalar.activation(
                                dst[:, (h0 + j) * NQ : (h0 + j + 1) * NQ],
                                ps[Dh * j : Dh * (j + 1), :],
                                mybir.ActivationFunctionType.Identity,
                                bias=bqk_sb[m][Dh * j : Dh * (j + 1), :],
                            )
                    # v seq-major: out [128 s, 256 dv] per s-tile
                    for st in range(4 * n, 4 * n + 4):
                        ps = mpp.tile([P, HPC * Dh], f32, tag="mm", name="psv")
                        for k in range(KD):
                            nc.tensor.matmul(
                                ps[:],
                                xb[k][:, P * (st % 4) : P * (st % 4 + 1)],
                                wq_sb[k][:, 2 * HPC * Dh : 3 * HPC * Dh],
                                start=(k == 0),
                                stop=(k == KD - 1),
                            )
                        vdst = v_sb[st][:].rearrange(
                            "p (h c) -> p h c", c=Dh + 1
                        )
                        nc.vector.tensor_tensor(
                            vdst[:, :, 0:Dh],
                            ps[:].rearrange("p (h c) -> p h c", c=Dh),
                            bvb_sb[:].rearrange("p (h c) -> p h c", c=Dh),
                            mybir.AluOpType.add,
                        )
                        nc.vector.tensor_copy(
                            vdst[:, :, Dh : Dh + 1], ones4f[:]
                        )
                        # stream out the present-v part
                        nc.sync.dma_start(
                            vp[P * st : P * (st + 1), :], vdst[:, :, 0:Dh]
                        )
                    # stream out the present-k part for this block
                    nc.sync.dma_start(
                        kp[:].rearrange("d (h s) -> d h s", s=S)[
                            :, :, NQ * n : NQ * (n + 1)
                        ],
                        kT_sb[n][:].rearrange("d (h s) -> d h s", s=NQ),
                    )

            # phase-2-only residents, loaded while qkv still computes
            mask_sb = res.tile([P, 4 * NQ], bf16, tag="mask")
            for t in range(4):
                nc.sync.dma_start(mask_sb[:, NQ * t : NQ * (t + 1)], cmask[t])
            wp_sb = []
            for k in range(2):
                wptmp = wrk.tile([P, D], f32, tag="wptmp", name="wptmp", bufs=2)
                nc.sync.dma_start(wptmp[:], wp[P * k : P * (k + 1), :])
                t = res.tile([P, D], bf16, tag=f"wp{k}", name=f"wp{k}")
                nc.vector.tensor_copy(t[:], wptmp[:])
                wp_sb.append(t)
            bpb_sb = res.tile([P, D], bf16, tag="bpb")
            nc.sync.dma_start(bpb_sb[:], bpb[:])

            # ---------- phase 2: attention + c_proj + reduce-scatter ----------
            for qb in range(NB):
                for h in range(HPC):
                    o_ps = opp.tile([P, NQ], f32, tag="o", name="o_ps")
                    nkt = 4 * qb + 4
                    for kt in range(nkt):
                        t = kt - 4 * qb
                        # diagonal tiles only touch q columns >= 128*t
                        c0 = P * t if t > 0 else 0
                        w = NQ - c0
                        st_ps = stp.tile([P, NQ], f32, tag="st", name="st_ps")
                        nc.tensor.matmul(
                            st_ps[:, 0:w],
                            kT_sb[kt // 4][
                                :, h * NQ + P * (kt % 4) : h * NQ + P * (kt % 4 + 1)
                            ],
                            qT_sb[qb][:, h * NQ + c0 : (h + 1) * NQ],
                            start=True,
                            stop=True,
                        )
                        st_sb = wrk.tile(
                            [P, NQ], bf16, tag="stsb", name="st_sb"
                        )
                        nc.scalar.activation(
                            st_sb[:, 0:w],
                            st_ps[:, 0:w],
                            mybir.ActivationFunctionType.Exp,
                            scale=0.125,
                        )
                        if t >= 0:
                            nc.vector.tensor_tensor(
                                st_sb[:, 0:w],
                                st_sb[:, 0:w],
                                mask_sb[:, NQ * t + c0 : NQ * (t + 1)],
                                mybir.AluOpType.mult,
                            )
                        nc.tensor.matmul(
                            o_ps[0 : Dh + 1, c0:NQ],
                            v_sb[kt][:, h * (Dh + 1) : (h + 1) * (Dh + 1)],
                            st_sb[:, 0:w],
                            start=(kt == 0),
                            stop=(kt == nkt - 1),
                        )
                    # normalize: attnT[h] = o[0:64] / l  (l = row 64 of o)
                    l_sb = wrk.tile([1, NQ], f32, tag="lsb", name="l_sb")
                    nc.vector.tensor_copy(l_sb[:], o_ps[Dh : Dh + 1, :])
                    linvf = wrk.tile([1, NQ], f32, tag="linvf", name="linvf")
                    nc.vector.reciprocal_approx_fast(linvf[:], l_sb[:])
                    linv = wrk.tile([1, NQ], f32r, tag="linv", name="linv")
                    with nc.allow_low_precision(reason="18-bit linv, bf16 out"):
                        nc.vector.tensor_copy(linv[:], linvf[:])
                    b_ps = mpp.tile([P, NQ], f32, tag="mm", name="b_ps")
                    nc.tensor.matmul(
                        b_ps[0:Dh, :],
                        ones64[:],
                        linv[:],
                        start=True,
                        stop=True,
                    )
                    bc_sb = wrk.tile([Dh, NQ], f32, tag="bc", name="bc_sb")
                    nc.vector.tensor_copy(bc_sb[:], b_ps[0:Dh, :])
                    nc.vector.tensor_tensor(
                        attnT_sb[h // 2][Dh * (h % 2) : Dh * (h % 2 + 1), :],
                        o_ps[0:Dh, :],
                        bc_sb[:],
                        mybir.AluOpType.mult,
                    )
                # c_proj partial for this q block
                for m in range(4):
                    for half in range(2):
                        ps = mpp.tile([P, NQ], f32, tag="mm", name="ps_cp")
                        for kt2 in range(2):
                            nc.tensor.matmul(
                                ps[:],
                                attnT_sb[kt2][:, P * m : P * (m + 1)],
                                wp_sb[kt2][:, NQ * half : NQ * (half + 1)],
                                start=(kt2 == 0),
                                stop=(kt2 == 1),
                            )
                        ap_sb = wrk.tile([P, NQ], bf16, tag="ap", name="ap_sb")
                        nc.vector.tensor_copy(ap_sb[:], ps[:])
                        nc.sync.dma_start(
                            cc_in[qb][
                                P * m : P * (m + 1), NQ * half : NQ * (half + 1)
                            ],
                            ap_sb[:],
                        )
                nc.gpsimd.collective_compute(
                    "ReduceScatter",
                    mybir.AluOpType.add,
                    ins=[cc_in[qb][:]],
                    outs=[cc_out[qb][:]],
                    replica_groups=GROUPS,
                )
                rs_sb = wrk.tile([P, D], bf16, tag="rs", name="rs_sb", bufs=2)
                nc.sync.dma_start(rs_sb[:], cc_out[qb][:])
                ao_sb = wrk.tile([P, D], f32, tag="ao", name="ao_sb", bufs=2)
                nc.vector.tensor_tensor(
                    ao_sb[:], rs_sb[:], bpb_sb[:], mybir.AluOpType.add
                )
                nc.sync.dma_start(a_out[qb], ao_sb[:])

    nc.compile()
    return nc


def shard_inputs(x, w_attn, b_attn, w_proj, b_proj):
    """Build the 8 per-core input maps from full inputs."""
    import ml_dtypes

    bf = ml_dtypes.bfloat16
    x = np.asarray(x, dtype=np.float32)
    w_attn = np.asarray(w_attn, dtype=np.float32)
    b_attn = np.asarray(b_attn, dtype=np.float32)
    w_proj = np.asarray(w_proj, dtype=np.float32)
    b_proj = np.asarray(b_proj, dtype=np.float32)

    # causal masks for the 4 diagonal k-tiles of each 512-wide q block
    i = np.arange(P)[:, None]
    j = np.arange(NQ)[None, :]
    cmask = np.stack([(j >= (P * t + i)).astype(bf) for t in range(4)])
    bpb = np.tile(b_proj[None, :], (P, 1)).astype(bf)

    in_maps = []
    for c in range(8):
        b = c // 4
        g = c % 4
        cols = slice(HPC * Dh * g, HPC * Dh * (g + 1))  # 256 cols of this core
        wq = w_attn[:, 0 * D : 1 * D][:, cols]
        wk = w_attn[:, 1 * D : 2 * D][:, cols]
        wv = w_attn[:, 2 * D : 3 * D][:, cols]
        bq = b_attn[0 * D : 1 * D][cols]
        bk = b_attn[1 * D : 2 * D][cols]
        bv = b_attn[2 * D : 3 * D][cols]
        in_maps.append(
            {
                "xT": np.ascontiguousarray(x[b].T),
                "wqkv": np.ascontiguousarray(
                    np.concatenate([wq, wk, wv], axis=1)
                ),
                "bqk": np.concatenate([bq, bk])[:, None].astype(np.float32),
                "bvb": np.tile(bv[None, :], (P, 1)).astype(np.float32),
                "wp": np.ascontiguousarray(w_proj[cols, :]),
                "bpb": bpb,
                "cmask": cmask,
            }
        )
    return in_maps


def assemble_outputs(results):
    """results: list of 8 per-core {kp, vp, a_out} -> (a, present)."""
    a = np.empty((B, S, D), dtype=np.float32)
    k = np.empty((B, H, S, Dh), dtype=np.float32)
    v = np.empty((B, H, S, Dh), dtype=np.float32)
    for c in range(8):
        b = c // 4
        g = c % 4
        kp = np.asarray(results[c]["kp"], dtype=np.float32)  # [64, HPC*S]
        vp = np.asarray(results[c]["vp"], dtype=np.float32)  # [S, HPC*64]
        for j in range(HPC):
            k[b, HPC * g + j] = kp[:, S * j : S * (j + 1)].T
            v[b, HPC * g + j] = vp[:, Dh * j : Dh * (j + 1)]
        ao = results[c]["a_out"]  # [NB, 128, D]
        for qb in range(NB):
            r0 = NQ * qb + P * g
            a[b, r0 : r0 + P] = ao[qb]
    present = np.stack([k, v])
    return a, present


def _get_nc():
    if "nc" not in _CACHE:
        _CACHE["nc"] = build_kernel()
    return _CACHE["nc"]


def kernel(x, w_attn, b_attn, w_proj, b_proj):
    from concourse.bass_utils import run_bass_kernel_spmd

    nc = _get_nc()
    in_maps = shard_inputs(x, w_attn, b_attn, w_proj, b_proj)
    res = run_bass_kernel_spmd(nc, in_maps, core_ids=list(range(8)))
    return assemble_outputs(res.results)


# revision 14
# speedup vs baseline: 1.2115x; 1.0834x over previous
"""Distributed causal multi-head attention (GPT-2 style Attention block)
for 8 Trainium2 NeuronCores.

Problem (hardcoded shapes): B=2, S=2048, D=1024, H=16 heads, Dh=64, f32.
reference computes:
    qkv = x @ w_attn + b_attn ; split q,k,v ; heads
    w = softmax(causal_mask(q k^T / 8))
    a = (w v) merged @ w_proj + b_proj
    present = stack(k, v)   # [2, B, H, S, Dh]

Sharding: data + head parallel. Core c handles batch b=c//4 and heads
H_c = [4*(c%4), 4*(c%4)+4). Each core:
  - computes q^T,k^T (head-dim-major) and v (seq-major) for its heads
  - full causal attention for its 4 heads over all S (identical static
    structure on every core -> one SPMD graph)
  - c_proj partial product with its 256 rows of w_proj
  - ReduceScatter(add) over its 4-core batch group, chunked by 512-row
    blocks of S so comm overlaps the remaining attention compute.
Matmul operands are bf16 (fast weight load + 2 elem/cycle streaming);
accumulation stays f32 in PSUM; softmax statistics stay f32.

kernel(**inputs) takes the FULL unsharded inputs and returns the full
(a, present) pair like the reference.
"""

import numpy as np

P = 128
B, S, D = 2, 2048, 1024
H = 16
Dh = 64
HPC = 4  # heads per core
NQ = 512  # q-block width
NB = S // NQ  # 4 q blocks
KT = S // P  # 16 k-tiles
KD = D // P  # 8 contraction tiles over D
GROUPS = [[0, 1, 2, 3], [4, 5, 6, 7]]

_CACHE = {}


def build_kernel():
    import concourse.mybir as mybir
    import concourse.tile as tile
    from concourse import bacc

    f32 = mybir.dt.float32
    f32r = mybir.dt.float32r
    bf16 = mybir.dt.bfloat16

    nc = bacc.Bacc(None, target_bir_lowering=False, num_devices=8)

    # ---- per-core external inputs ----
    xT = nc.dram_tensor("xT", [D, S], f32, kind="ExternalInput")
    wqkv = nc.dram_tensor("wqkv", [D, 3 * HPC * Dh], f32, kind="ExternalInput")
    bqk = nc.dram_tensor("bqk", [2 * HPC * Dh, 1], f32, kind="ExternalInput")
    bvb = nc.dram_tensor("bvb", [P, HPC * Dh], f32, kind="ExternalInput")
    wp = nc.dram_tensor("wp", [HPC * Dh, D], f32, kind="ExternalInput")
    bpb = nc.dram_tensor("bpb", [P, D], bf16, kind="ExternalInput")
    cmask = nc.dram_tensor("cmask", [4, P, NQ], bf16, kind="ExternalInput")

    # ---- per-core outputs ----
    kp = nc.dram_tensor("kp", [Dh, HPC * S], bf16, kind="ExternalOutput")
    vp = nc.dram_tensor("vp", [S, HPC * Dh], bf16, kind="ExternalOutput")
    a_out = nc.dram_tensor("a_out", [NB, P, D], f32, kind="ExternalOutput")

    # ---- internal DRAM for the chunked reduce-scatter ----
    cc_in = [
        nc.dram_tensor(f"cc_in{qb}", [NQ, D], bf16, kind="Internal")
        for qb in range(NB)
    ]
    cc_out = [
        nc.dram_tensor(f"cc_out{qb}", [P, D], bf16, kind="Internal")
        for qb in range(NB)
    ]

    with tile.TileContext(nc) as tc:
        with (
            tc.tile_pool(name="res", bufs=1) as res,  # whole-kernel residents
            tc.tile_pool(name="wrk", bufs=3) as wrk,  # rotating staging
            tc.tile_pool(name="stp", bufs=4, space="PSUM") as stp,
            tc.tile_pool(name="opp", bufs=2, space="PSUM") as opp,
            tc.tile_pool(name="mpp", bufs=2, space="PSUM") as mpp,
        ):
            # ---------- whole-kernel resident tensors ----------
            bqk_sb = []
            for m in range(4):
                t = res.tile([P, 1], f32, tag=f"bqk{m}", name=f"bqk{m}")
                nc.sync.dma_start(t[:], bqk[P * m : P * (m + 1), :])
                bqk_sb.append(t)
            bvb_sb = res.tile([P, HPC * Dh], f32, tag="bvb")
            nc.sync.dma_start(bvb_sb[:], bvb[:])
            ones64f = res.tile([1, Dh], f32, tag="ones64f")
            nc.vector.memset(ones64f[:], 1.0)
            ones64 = res.tile([1, Dh], f32r, tag="ones64")
            nc.vector.tensor_copy(ones64[:], ones64f[:])
            ones4f = res.tile([P, HPC], f32, tag="ones4f")
            nc.vector.memset(ones4f[:], 1.0)

            # q^T / k^T head-dim-major, per 512-col block:
            # qT_sb[n] is [64, HPC*NQ]; head h occupies cols [h*NQ,(h+1)*NQ)
            qT_sb = [
                res.tile([Dh, HPC * NQ], bf16, tag=f"qT{n}", name=f"qT{n}")
                for n in range(NB)
            ]
            kT_sb = [
                res.tile([Dh, HPC * NQ], bf16, tag=f"kT{n}", name=f"kT{n}")
                for n in range(NB)
            ]
            # v seq-major with a ones column per head: [128, HPC*65] per s-tile
            v_sb = [
                res.tile([P, HPC * (Dh + 1)], bf16, tag=f"v{st}", name=f"v{st}")
                for st in range(KT)
            ]
            # attention output^T (d-major) for the current q block
            attnT_sb = [
                res.tile([P, NQ], bf16, tag=f"attnT{t}", name=f"attnT{t}")
                for t in range(2)
            ]

            # ---------- qkv projection, one 512-column block at a time ----
            # (block n is emitted just before attention q-block n-1 so the
            # scheduler can overlap projection matmuls with the exp-paced
            # attention chain and keep the PE array warm)
            ph1 = tc.tile_pool(name="ph1", bufs=2)
            ph1_pool = ph1.__enter__()
            wq_sb = []
            for k in range(KD):
                wtmp = ph1_pool.tile(
                    [P, 3 * HPC * Dh], f32, tag="wtmp", name="wtmp", bufs=4
                )
                nc.sync.dma_start(wtmp[:], wqkv[P * k : P * (k + 1), :])
                w = ph1_pool.tile(
                    [P, 3 * HPC * Dh],
                    bf16,
                    tag=f"w{k}",
                    name=f"w{k}",
                    bufs=1,
                )
                nc.scalar.activation(
                    w[:], wtmp[:], mybir.ActivationFunctionType.Copy
                )
                wq_sb.append(w)

            def qkv_block(n):
                xb = []
                for k in range(KD):
                    xtmp = ph1_pool.tile(
                        [P, NQ], f32, tag="xtmp", name="xtmp", bufs=3
                    )
                    nc.sync.dma_start(
                        xtmp[:],
                        xT[P * k : P * (k + 1), NQ * n : NQ * (n + 1)],
                    )
                    t = ph1_pool.tile(
                        [P, NQ], bf16, tag=f"x{k}", name=f"x{k}", bufs=1
                    )
                    nc.vector.tensor_copy(t[:], xtmp[:])
                    xb.append(t)
                # q,k d-major: m 0,1 -> q heads {0,1},{2,3}; m 2,3 -> k
                for m in range(4):
                    dst = qT_sb[n] if m < 2 else kT_sb[n]
                    h0 = 2 * (m % 2)
                    ps = mpp.tile([P, NQ], f32, tag="mm", name="psqk")
                    for k in range(KD):
                        nc.tensor.matmul(
                            ps[:],
                            wq_sb[k][:, P * m : P * (m + 1)],
                            xb[k][:],
                            start=(k == 0),
                            stop=(k == KD - 1),
                        )
                    for j in range(2):
                        nc.scalar.activation(
                            dst[:, (h0 + j) * NQ : (h0 + j + 1) * NQ],
                            ps[Dh * j : Dh * (j + 1), :],
                            mybir.ActivationFunctionType.Identity,
                            bias=bqk_sb[m][Dh * j : Dh * (j + 1), :],
                        )
                # v seq-major: out [128 s, 256 dv] per s-tile
                for st in range(4 * n, 4 * n + 4):
                    ps = mpp.tile([P, HPC * Dh], f32, tag="mm", name="psv")
                    for k in range(KD):
                        nc.tensor.matmul(
                            ps[:],
                            xb[k][:, P * (st % 4) : P * (st % 4 + 1)],
                            wq_sb[k][:, 2 * HPC * Dh : 3 * HPC * Dh],
                            start=(k == 0),
                            stop=(k == KD - 1),
                        )
                    vdst = v_sb[st][:].rearrange("p (h c) -> p h c", c=Dh + 1)
                    nc.vector.tensor_tensor(
                        vdst[:, :, 0:Dh],
                        ps[:].rearrange("p (h c) -> p h c", c=Dh),
                        bvb_sb[:].rearrange("p (h c) -> p h c", c=Dh),
                        mybir.AluOpType.add,
                    )
                    nc.vector.tensor_copy(vdst[:, :, Dh : Dh + 1], ones4f[:])
                    # stream out the present-v part
                    nc.sync.dma_start(
                        vp[P * st : P * (st + 1), :], vdst[:, :, 0:Dh]
                    )
                # stream out the present-k part for this block
                nc.sync.dma_start(
                    kp[:].rearrange("d (h s) -> d h s", s=S)[
                        :, :, NQ * n : NQ * (n + 1)
                    ],
                    kT_sb[n][:].rearrange("d (h s) -> d h s", s=NQ),
                )

            qkv_block(0)
            # phase-2-only residents, loaded while qkv still computes
            mask_sb = res.tile([P, 4 * NQ], bf16, tag="mask")
            for t in range(4):
                nc.sync.dma_start(mask_sb[:, NQ * t : NQ * (t + 1)], cmask[t])
            wp_sb = []
            for k in range(2):
                wptmp = wrk.tile([P, D], f32, tag="wptmp", name="wptmp", bufs=2)
                nc.sync.dma_start(wptmp[:], wp[P * k : P * (k + 1), :])
                t = res.tile([P, D], bf16, tag=f"wp{k}", name=f"wp{k}")
                nc.vector.tensor_copy(t[:], wptmp[:])
                wp_sb.append(t)
            bpb_sb = res.tile([P, D], bf16, tag="bpb")
            nc.sync.dma_start(bpb_sb[:], bpb[:])

            # ---------- phase 2: attention + c_proj + reduce-scatter ----------
            for qb in range(NB):
                if qb + 1 < NB:
                    qkv_block(qb + 1)
                if qb + 1 == NB:
                    ph1.__exit__(None, None, None)
                nkt = 4 * qb + 4
                for hp in range(2):
                    o_list = {}
                    for h in (2 * hp, 2 * hp + 1):
                        o_list[h] = opp.tile(
                            [P, NQ], f32, tag="o", name="o_ps", bufs=2
                        )
                    for kt in range(nkt):
                        t = kt - 4 * qb
                        # diagonal tiles only touch q columns >= 128*t
                        c0 = P * t if t > 0 else 0
                        w = NQ - c0
                        for h in (2 * hp, 2 * hp + 1):
                            o_ps = o_list[h]
                            st_ps = stp.tile(
                                [P, NQ], f32, tag="st", name="st_ps"
                            )
                            nc.tensor.matmul(
                                st_ps[:, 0:w],
                                kT_sb[kt // 4][
                                    :,
                                    h * NQ
                                    + P * (kt % 4) : h * NQ
                                    + P * (kt % 4 + 1),
                                ],
                                qT_sb[qb][:, h * NQ + c0 : (h + 1) * NQ],
                                start=True,
                                stop=True,
                            )
                            st_sb = wrk.tile(
                                [P, NQ], bf16, tag="stsb", name="st_sb", bufs=4
                            )
                            nc.scalar.activation(
                                st_sb[:, 0:w],
                                st_ps[:, 0:w],
                                mybir.ActivationFunctionType.Exp,
                                scale=0.125,
                            )
                            if t >= 0:
                                nc.vector.tensor_tensor(
                                    st_sb[:, 0:w],
                                    st_sb[:, 0:w],
                                    mask_sb[:, NQ * t + c0 : NQ * (t + 1)],
                                    mybir.AluOpType.mult,
                                )
                            nc.tensor.matmul(
                                o_ps[0 : Dh + 1, c0:NQ],
                                v_sb[kt][
                                    :, h * (Dh + 1) : (h + 1) * (Dh + 1)
                                ],
                                st_sb[:, 0:w],
                                start=(kt == 0),
                                stop=(kt == nkt - 1),
                            )
                    for h in (2 * hp, 2 * hp + 1):
                        o_ps = o_list[h]
                        # normalize: attnT[h] = o[0:64] / l (l = row 64 of o)
                        l_sb = wrk.tile([1, NQ], f32, tag="lsb", name="l_sb")
                        nc.vector.tensor_copy(l_sb[:], o_ps[Dh : Dh + 1, :])
                        linvf = wrk.tile(
                            [1, NQ], f32, tag="linvf", name="linvf"
                        )
                        nc.vector.reciprocal_approx_fast(linvf[:], l_sb[:])
                        linv = wrk.tile(
                            [1, NQ], f32r, tag="linv", name="linv"
                        )
                        with nc.allow_low_precision(
                            reason="18-bit linv, bf16 out"
                        ):
                            nc.vector.tensor_copy(linv[:], linvf[:])
                        b_ps = mpp.tile([P, NQ], f32, tag="mm", name="b_ps")
                        nc.tensor.matmul(
                            b_ps[0:Dh, :],
                            ones64[:],
                            linv[:],
                            start=True,
                            stop=True,
                        )
                        bc_sb = wrk.tile([Dh, NQ], f32, tag="bc", name="bc_sb")
                        nc.vector.tensor_copy(bc_sb[:], b_ps[0:Dh, :])
                        nc.vector.tensor_tensor(
                            attnT_sb[h // 2][
                                Dh * (h % 2) : Dh * (h % 2 + 1), :
                            ],
                            o_ps[0:Dh, :],
                            bc_sb[:],
                            mybir.AluOpType.mult,
                        )
                # c_proj partial for this q block
                for m in range(4):
                    for half in range(2):
                        ps = mpp.tile([P, NQ], f32, tag="mm", name="ps_cp")
                        for kt2 in range(2):
                            nc.tensor.matmul(
                                ps[:],
                                attnT_sb[kt2][:, P * m : P * (m + 1)],
                                wp_sb[kt2][:, NQ * half : NQ * (half + 1)],
                                start=(kt2 == 0),
                                stop=(kt2 == 1),
                            )
                        ap_sb = wrk.tile([P, NQ], bf16, tag="ap", name="ap_sb")
                        nc.vector.tensor_copy(ap_sb[:], ps[:])
                        nc.sync.dma_start(
                            cc_in[qb][
                                P * m : P * (m + 1), NQ * half : NQ * (half + 1)
                            ],
                            ap_sb[:],
                        )
                nc.gpsimd.collective_compute(
                    "ReduceScatter",
                    mybir.AluOpType.add,
                    ins=[cc_in[qb][:]],
                    outs=[cc_out[qb][:]],
                    replica_groups=GROUPS,
                )
                rs_sb = wrk.tile([P, D], bf16, tag="rs", name="rs_sb", bufs=2)
                nc.sync.dma_start(rs_sb[:], cc_out[qb][:])
                ao_sb = wrk.tile([P, D], f32, tag="ao", name="ao_sb", bufs=2)
                nc.vector.tensor_tensor(
                    ao_sb[:], rs_sb[:], bpb_sb[:], mybir.AluOpType.add
                )
                nc.sync.dma_start(a_out[qb], ao_sb[:])

    nc.compile()
    return nc


def shard_inputs(x, w_attn, b_attn, w_proj, b_proj):
    """Build the 8 per-core input maps from full inputs."""
    import ml_dtypes

    bf = ml_dtypes.bfloat16
    x = np.asarray(x, dtype=np.float32)
    w_attn = np.asarray(w_attn, dtype=np.float32)
    b_attn = np.asarray(b_attn, dtype=np.float32)
    w_proj = np.asarray(w_proj, dtype=np.float32)
    b_proj = np.asarray(b_proj, dtype=np.float32)

    # causal masks for the 4 diagonal k-tiles of each 512-wide q block
    i = np.arange(P)[:, None]
    j = np.arange(NQ)[None, :]
    cmask = np.stack([(j >= (P * t + i)).astype(bf) for t in range(4)])
    bpb = np.tile(b_proj[None, :], (P, 1)).astype(bf)

    in_maps = []
    for c in range(8):
        b = c // 4
        g = c % 4
        cols = slice(HPC * Dh * g, HPC * Dh * (g + 1))  # 256 cols of this core
        wq = w_attn[:, 0 * D : 1 * D][:, cols]
        wk = w_attn[:, 1 * D : 2 * D][:, cols]
        wv = w_attn[:, 2 * D : 3 * D][:, cols]
        bq = b_attn[0 * D : 1 * D][cols]
        bk = b_attn[1 * D : 2 * D][cols]
        bv = b_attn[2 * D : 3 * D][cols]
        in_maps.append(
            {
                "xT": np.ascontiguousarray(x[b].T),
                "wqkv": np.ascontiguousarray(
                    np.concatenate([wq, wk, wv], axis=1)
                ),
                "bqk": np.concatenate([bq, bk])[:, None].astype(np.float32),
                "bvb": np.tile(bv[None, :], (P, 1)).astype(np.float32),
                "wp": np.ascontiguousarray(w_proj[cols, :]),
                "bpb": bpb,
                "cmask": cmask,
            }
        )
    return in_maps


def assemble_outputs(results):
    """results: list of 8 per-core {kp, vp, a_out} -> (a, present)."""
    a = np.empty((B, S, D), dtype=np.float32)
    k = np.empty((B, H, S, Dh), dtype=np.float32)
    v = np.empty((B, H, S, Dh), dtype=np.float32)
    for c in range(8):
        b = c // 4
        g = c % 4
        kp = np.asarray(results[c]["kp"], dtype=np.float32)  # [64, HPC*S]
        vp = np.asarray(results[c]["vp"], dtype=np.float32)  # [S, HPC*64]
        for j in range(HPC):
            k[b, HPC * g + j] = kp[:, S * j : S * (j + 1)].T
            v[b, HPC * g + j] = vp[:, Dh * j : Dh * (j + 1)]
        ao = results[c]["a_out"]  # [NB, 128, D]
        for qb in range(NB):
            r0 = NQ * qb + P * g
            a[b, r0 : r0 + P] = ao[qb]
    present = np.stack([k, v])
    return a, present


def _get_nc():
    if "nc" not in _CACHE:
        _CACHE["nc"] = build_kernel()
    return _CACHE["nc"]


def kernel(x, w_attn, b_attn, w_proj, b_proj):
    from concourse.bass_utils import run_bass_kernel_spmd

    nc = _get_nc()
    in_maps = shard_inputs(x, w_attn, b_attn, w_proj, b_proj)
    res = run_bass_kernel_spmd(nc, in_maps, core_ids=list(range(8)))
    return assemble_outputs(res.results)
